# revision 1
# baseline (speedup 1.0000x reference)
"""Deformable attention Trainium2 kernel (8-core SPMD).

Sharding: core c -> batch b=c//4, output row block R0=16*(c%4) (16 rows x 64
cols = 1024 px). Each core computes its (b, rows) slice of the full output for
all heads, so no cross-core communication (the MLP mixes channels, not pixels).
k/v are projected over a 48-row halo; bilinear samples outside it are zeroed by
validity weights (offsets ~N(0,1); |off|>14 cannot occur).

Per (g,t) image (24 per core): 24 k-ch (+8 pad) live in a 49x97 zero-bordered
canvas; 4 images stack into a 128-partition quad. GPSIMD ap_gather fetches the
4 bilinear corners; q.k channel reduction and coefficient replication run on
the PE via 0/1 selector matmuls; bilinear weights / softmax / weighted-v
reduction run on DVE/ACT in [24 img, sample] planes. MLP uses exact erf-gelu.
"""

import sys

sys.path.insert(0, "/opt/trn_rl_repo")

import contextlib

import numpy as np
import ml_dtypes

import concourse.bass as bass
import concourse.mybir as mybir
import concourse.tile as tile
from concourse import bacc
from concourse.bass_utils import run_bass_kernel_spmd

F32 = mybir.dt.float32
F32R = mybir.dt.float32r
F16 = mybir.dt.float16
BF = mybir.dt.bfloat16
I16 = mybir.dt.int16
I32 = mybir.dt.int32
AL = mybir.AluOpType
ACTF = mybir.ActivationFunctionType
AX = mybir.AxisListType

B, C, H, W = 2, 288, 64, 64
T, G, K = 2, 12, 9
HD = C // G  # 24
RB, PX = 16, 16 * 64  # rows / pixels per core
NS = PX * K  # samples per image (px-major: (px, tap))
CR, CC = 49, 97
CN = CR * CC  # canvas cells (4753)
HALO = 48
SCALE = float(HD) ** -0.5
NCH = 16  # sample chunks per image
CH = NS // NCH  # 576 samples
CHPX = PX // NCH  # 64 px
NW = CH // 16  # wrapped idx cols per chunk

_CACHE = {}


def build_program():
    nc = bacc.Bacc("TRN2", target_bir_lowering=False, debug=False)

    def din(name, shape, dt=F32):
        return nc.dram_tensor(name, list(shape), dt, kind="ExternalInput").ap()

    io = {}
    io["q_in"] = din("q_in", (C, PX), BF)
    io["k_in"] = din("k_in", (T, C, HALO * W), BF)
    io["v_in"] = din("v_in", (T, C, HALO * W), BF)
    io["off_in"] = din("off_in", (128, NS))
    io["base_in"] = din("base_in", (128, NS))
    io["csub_in"] = din("csub_in", (128, 1))
    io["chi_in"] = din("chi_in", (128, 1))
    io["wqt"] = din("wqt", (C, C), BF)
    io["wkt"] = din("wkt", (C, C), BF)
    io["wvt"] = din("wvt", (C, C), BF)
    io["w1t"] = din("w1t", (C, 2 * C), BF)
    io["w2t"] = din("w2t", (2 * C, C), BF)
    io["bqs"] = din("bqs", (C, 1))  # bq * SCALE
    io["bkvq"] = din("bkvq", (96, 6))  # quad bias cols: (which k=0/v=1)*3 + qd3
    io["b1"] = din("b1", (2 * C, 1))
    io["b2"] = din("b2", (C, 1))
    io["sel4"] = din("sel4", (128, 4), BF)
    io["selrep"] = din("selrep", (64, 3 * 128), F16)  # per qd3: col p -> group row
    io["selv4"] = din("selv4", (128, 4 * HD), BF)
    io["L4_d"] = nc.dram_tensor("L4_d", [64, 4 * NS], F16).ap()
    io["out_d"] = nc.dram_tensor("out", [C, PX], F32, kind="ExternalOutput").ap()
    io["oatt_d"] = nc.dram_tensor("oatt_d", [128, 3 * PX], F32, kind="ExternalOutput").ap()

    with tile.TileContext(nc) as tc:
        _body(tc, nc, io)
    nc.compile()
    return nc


def _dma_to_chrows(sync, dst_tile, px, src_ap, ch0):
    """DMA src [24, px] into channel rows ch0..ch0+24 of a [128, 3*px] layout
    tile (ch c -> (c%128, c//128)), splitting at 128 boundaries."""
    lo, hi = ch0, ch0 + 24
    while lo < hi:
        kk = lo // 128
        r0 = lo - 128 * kk
        n = min(hi - lo, 128 - r0)
        s0 = lo - ch0
        sync.dma_start(
            out=dst_tile[r0 : r0 + n, kk * px : (kk + 1) * px],
            in_=src_ap[s0 : s0 + n, :],
        )
        lo += n


def _body(tc, nc, io):
    dve, act, gps, pe, sync = nc.vector, nc.scalar, nc.gpsimd, nc.tensor, nc.sync
    es = contextlib.ExitStack()
    ect = es.enter_context
    ctx = ect(contextlib.ExitStack())

    def mm(out, lhsT, rhs, start, stop):
        n = out.shape[-1]
        assert rhs.shape[-1] == n
        for c0 in range(0, n, 512):
            c1 = min(c0 + 512, n)
            pe.matmul(
                out[..., c0:c1], lhsT, rhs[..., c0:c1], start=start, stop=stop
            )

    def btap(ap2d, n, k):  # [p, n] -> [p, n, k] broadcast view
        return ap2d.unsqueeze(-1).to_broadcast([ap2d.shape[0], n, k])

    sb = ect(tc.tile_pool(name="persist", bufs=1))

    # ---------------- weight/selector staging ----------------
    wk_s = sb.tile([128, 3 * C], BF, name="wk_s")
    wv_s = sb.tile([128, 3 * C], BF, name="wv_s")
    for i in range(3):
        n = min(128, C - 128 * i)
        sync.dma_start(out=wk_s[:n, i * C : (i + 1) * C], in_=io["wkt"][128 * i : 128 * i + n, :])
        sync.dma_start(out=wv_s[:n, i * C : (i + 1) * C], in_=io["wvt"][128 * i : 128 * i + n, :])
    bkvq_s = sb.tile([96, 6], F32, name="bkvq_s")
    sync.dma_start(out=bkvq_s[:], in_=io["bkvq"][:])
    sel4_s = sb.tile([128, 4], BF, name="sel4_s")
    sync.dma_start(out=sel4_s[:], in_=io["sel4"][:])
    selrep_s = sb.tile([64, 3 * 128], F16, name="selrep_s")
    sync.dma_start(out=selrep_s[:], in_=io["selrep"][:])
    selv4_s = sb.tile([128, 4 * HD], BF, name="selv4_s")
    sync.dma_start(out=selv4_s[:], in_=io["selv4"][:])
    wrp = sb.tile([128, 6 * (NS // 16)], I16, name="wrp")
    oatt = sb.tile([128, 3 * PX], F32, name="oatt")
    act.memzero(oatt[:])

    wes = contextlib.ExitStack()  # weights live: build .. coef4
    pw = wes.enter_context(tc.tile_pool(name="pw", bufs=1))
    p_wy0 = pw.tile([64, NS], F16, name="p_wy0")
    p_wy1 = pw.tile([64, NS], F16, name="p_wy1")
    p_wxi = pw.tile([64, 2 * NS], F16, name="p_wxi")  # (wx0,wx1) interleaved
    idx_dram = nc.dram_tensor("idx_dram", [64, NS], I16).ap()

    # ---------------- q projection (scaled, bias folded) ----------------
    qes = contextlib.ExitStack()
    qpool = qes.enter_context(tc.tile_pool(name="qrep_pool", bufs=1))
    qrep = []
    with tc.tile_pool(name="qph", bufs=2) as qsc, tc.tile_pool(
        name="qph_ps", bufs=2, space="PSUM"
    ) as qpp:
        wq_s = qsc.tile([128, 3 * C], BF, name="wq_s", tag="wq")
        bqs_s = qsc.tile([128, 3], F32, name="bqs_s", tag="bq")
        qp_s = qsc.tile([128, 3 * PX], F32, name="qp_s", tag="qp")
        for i in range(3):
            n = min(128, C - 128 * i)
            sync.dma_start(out=wq_s[:n, i * C : (i + 1) * C], in_=io["wqt"][128 * i : 128 * i + n, :])
            sync.dma_start(out=bqs_s[:n, i : i + 1], in_=io["bqs"][128 * i : 128 * i + n, :])
        for m in range(3):
            mn = min(128, C - 128 * m)
            for nch in range(PX // 512):
                ps = qpp.tile([128, 512], F32, name="qps", tag="qps")
                for kk in range(3):
                    kn = min(128, C - 128 * kk)
                    rhs = qsc.tile([128, 512], BF, name="qrhs", tag=f"qrhs{kk}")
                    sync.dma_start(
                        out=rhs[:kn, :],
                        in_=io["q_in"][128 * kk : 128 * kk + kn, nch * 512 : nch * 512 + 512],
                    )
                    mm(
                        ps[:mn, :],
                        wq_s[:kn, kk * C + 128 * m : kk * C + 128 * m + mn],
                        rhs[:kn, :],
                        start=(kk == 0),
                        stop=(kk == 2),
                    )
                act.activation(
                    qp_s[:mn, m * PX + nch * 512 : m * PX + nch * 512 + 512],
                    ps[:mn, :],
                    ACTF.Identity,
                    bias=bqs_s[:mn, m : m + 1],
                    scale=SCALE,
                )
        def qch(c0, n):  # list of (qp_s row-slice) covering ch c0..c0+n
            out = []
            lo = c0
            while lo < c0 + n:
                kk = lo // 128
                r0 = lo - 128 * kk
                cnt = min(c0 + n - lo, 128 - r0)
                out.append(qp_s[r0 : r0 + cnt, kk * PX : kk * PX + PX])
                lo += cnt
            return out

        for qd3 in range(3):
            qr = qpool.tile([128, PX], F32, name=f"qrep{qd3}")
            for j in range(4):
                g = 4 * qd3 + j
                r = 32 * j
                for piece in qch(24 * g, 24):
                    np_ = piece.shape[0]
                    sync.dma_start(out=qr[r : r + np_, :], in_=piece)
                    r += np_
                for piece in qch(24 * g, 8):
                    np_ = piece.shape[0]
                    sync.dma_start(out=qr[r : r + np_, :], in_=piece)
                    r += np_
            qrep.append(qr)

    # ---------------- offsets -> bilinear weights + wrapped indices ----------
    QN = NS // 8
    with tc.tile_pool(name="wb", bufs=1) as wb:
        for qq in range(8):
            cs = slice(qq * QN, (qq + 1) * QN)
            offp = wb.tile([128, QN], F32, name="offp", tag="offp")
            basep = wb.tile([128, QN], F32, name="basep", tag="basep")
            sync.dma_start(out=offp[:], in_=io["off_in"][:, cs])
            sync.dma_start(out=basep[:], in_=io["base_in"][:, cs])
            csubp = wb.tile([128, 1], F32, name="csubp", tag="csubp")
            chip = wb.tile([128, 1], F32, name="chip", tag="chip")
            sync.dma_start(out=csubp[:], in_=io["csub_in"][:])
            sync.dma_start(out=chip[:], in_=io["chi_in"][:])
            pos = wb.tile([128, QN], F32, name="pos", tag="pos")
            ii = wb.tile([128, QN], I32, name="ii", tag="ii")
            flo = wb.tile([128, QN], F32, name="flo", tag="flo")
            ta = wb.tile([128, QN], F32, name="ta", tag="ta")
            tb = wb.tile([128, QN], F32, name="tb", tag="tb")
            tg = wb.tile([128, QN], F32, name="tg", tag="tg")
            dve.tensor_tensor(out=pos[:], in0=offp[:], in1=basep[:], op=AL.add)
            dve.tensor_copy(out=ii[:], in_=pos[:])
            dve.tensor_copy(out=flo[:], in_=ii[:])
            dve.tensor_tensor(out=ta[:], in0=flo[:], in1=pos[:], op=AL.is_gt)
            dve.tensor_tensor(out=flo[:], in0=flo[:], in1=ta[:], op=AL.subtract)
            dve.tensor_tensor(out=ta[:], in0=pos[:], in1=flo[:], op=AL.subtract)
            dve.tensor_scalar(out=tb[:], in0=flo[:], scalar1=64.0, scalar2=None, op0=AL.is_ge)
            dve.tensor_scalar(out=tg[:], in0=flo[:], scalar1=127.0, scalar2=None, op0=AL.is_le)
            dve.tensor_tensor(out=tb[:], in0=tb[:], in1=tg[:], op=AL.mult)
            dve.tensor_tensor(out=tg[:], in0=ta[:], in1=tb[:], op=AL.mult)
            dve.tensor_tensor(out=tb[:], in0=tb[:], in1=tg[:], op=AL.subtract)  # w0
            dve.tensor_copy(out=p_wy0[:, cs], in_=tb[:64, :])
            xsh = wb.tile([64, QN], F32, name="xsh", tag="xsh")
            sync.dma_start(out=xsh[:], in_=tb[64:128, :])
            dve.tensor_copy(
                out=p_wxi[:, 2 * qq * QN : 2 * (qq + 1) * QN].rearrange(
                    "p (n two) -> p n two", two=2
                )[:, :, 0],
                in_=xsh[:],
            )
            dve.tensor_scalar(out=tb[:], in0=flo[:], scalar1=63.0, scalar2=None, op0=AL.is_ge)
            dve.tensor_tensor(out=tb[:], in0=tb[:], in1=ta[:], op=AL.mult)
            dve.tensor_scalar(out=ta[:], in0=flo[:], scalar1=126.0, scalar2=None, op0=AL.is_le)
            dve.tensor_tensor(out=tb[:], in0=tb[:], in1=ta[:], op=AL.mult)  # w1
            dve.tensor_copy(out=p_wy1[:, cs], in_=tb[:64, :])
            xsh2 = wb.tile([64, QN], F32, name="xsh2", tag="xsh2")
            sync.dma_start(out=xsh2[:], in_=tb[64:128, :])
            dve.tensor_copy(
                out=p_wxi[:, 2 * qq * QN : 2 * (qq + 1) * QN].rearrange(
                    "p (n two) -> p n two", two=2
                )[:, :, 1],
                in_=xsh2[:],
            )
            dve.tensor_tensor(
                out=flo[:], in0=flo[:], in1=csubp[:].to_broadcast([128, QN]), op=AL.subtract
            )
            dve.tensor_scalar(out=flo[:], in0=flo[:], scalar1=0.0, scalar2=None, op0=AL.max)
            dve.tensor_tensor(
                out=flo[:], in0=flo[:], in1=chip[:].to_broadcast([128, QN]), op=AL.min
            )
            xsh3 = wb.tile([64, QN], F32, name="xsh3", tag="xsh3")
            sync.dma_start(out=xsh3[:], in_=flo[64:128, :])
            dve.tensor_scalar(
                out=ta[:64, :], in0=flo[:64, :], scalar1=float(CC), scalar2=None, op0=AL.mult
            )
            dve.tensor_tensor(out=ta[:64, :], in0=ta[:64, :], in1=xsh3[:], op=AL.add)
            i16 = wb.tile([64, QN], I16, name="i16", tag="i16")
            dve.tensor_copy(out=i16[:], in_=ta[:64, :])
            sync.dma_start(out=idx_dram[:, cs], in_=i16[:])
    for qd in range(6):
        for j in range(4):
            img = 32 * (qd // 3) + 4 * (qd % 3) + j
            sap = idx_dram[img : img + 1, :].rearrange("o (c p) -> (o p) c", p=16)
            sync.dma_start(
                out=wrp[32 * j : 32 * j + 16, qd * (NS // 16) : (qd + 1) * (NS // 16)], in_=sap
            )
            sync.dma_start(
                out=wrp[32 * j + 16 : 32 * j + 32, qd * (NS // 16) : (qd + 1) * (NS // 16)],
                in_=sap,
            )

    # ---------------- canvas construction ----------------
    def make_canvas(cvp, scp, cpp, which, qd):
        wmat = wk_s if which == 0 else wv_s
        src = io["k_in"] if which == 0 else io["v_in"]
        ti, qd3 = qd // 3, qd % 3
        canq = cvp.tile([128, CN], F32, name="canq", tag="canq")
        act.memzero(canq[:])
        for nch in range(6):
            ps = cpp.tile([96, 512], F32, name="cvps", tag="cvps")
            for kk in range(3):
                kn = min(128, C - 128 * kk)
                rhs = scp.tile([128, 512], BF, name="cvrhs", tag=f"cvrhs{kk}")
                sync.dma_start(
                    out=rhs[:kn, :],
                    in_=src[ti, 128 * kk : 128 * kk + kn, nch * 512 : nch * 512 + 512],
                )
                mm(
                    ps[:, :],
                    wmat[:kn, kk * C + 96 * qd3 : kk * C + 96 * qd3 + 96],
                    rhs[:kn, :],
                    start=(kk == 0),
                    stop=(kk == 2),
                )
            stg = scp.tile([96, 512], F32, name="cvstg", tag="cvstg")
            act.activation(
                stg[:, :], ps[:, :], ACTF.Identity, bias=bkvq_s[:, which * 3 + qd3 : which * 3 + qd3 + 1], scale=1.0
            )
            for j in range(4):
                dst = canq[32 * j : 32 * j + 24, :].rearrange("p (r c) -> p r c", r=CR)[
                    :, nch * 8 : nch * 8 + 8, 16:80
                ]
                sync.dma_start(
                    out=dst,
                    in_=stg[24 * j : 24 * j + 24, :].rearrange("p (r c) -> p r c", r=8),
                )
        return canq

    # ---------------- K phase ----------------
    with (
        tc.tile_pool(name="kcv", bufs=1) as kcv,
        tc.tile_pool(name="ksc", bufs=2) as ksc,
        tc.tile_pool(name="kpp", bufs=2, space="PSUM") as kpp,
    ):
        for qd in range(6):
            qd3 = qd % 3
            canq = make_canvas(kcv, ksc, kpp, 0, qd)
            for chunk in range(NCH):
                wsl = wrp[:, qd * (NS // 16) + chunk * NW : qd * (NS // 16) + (chunk + 1) * NW]
                l4t = ksc.tile([4, 4 * CH], F16, name="l4t", tag="l4t")
                l4v = l4t[:].rearrange("p (n four) -> p four n", four=4)
                for ci, dlt in enumerate((0, 1, CC, CC + 1)):
                    it = ksc.tile([128, NW], I16, name="it", tag="it")
                    dve.tensor_scalar(out=it[:], in0=wsl, scalar1=dlt, scalar2=None, op0=AL.add)
                    gt = ksc.tile([128, CH], F32, name="gt", tag="gt")
                    gps.ap_gather(gt[:], canq[:].unsqueeze(-1), it[:], 128, CN, 1, CH)
                    gtb = ksc.tile([128, CH], BF, name="gtb", tag="gtb")
                    dve.tensor_tensor(
                        out=gtb[:].rearrange("p (n k) -> p n k", k=K),
                        in0=gt[:].rearrange("p (n k) -> p n k", k=K),
                        in1=btap(qrep[qd3][:, chunk * CHPX : (chunk + 1) * CHPX], CHPX, K),
                        op=AL.mult,
                    )
                    lps = kpp.tile([4, CH], F32, name="lps", tag="lps")
                    mm(lps[:, :], sel4_s[:, :], gtb[:, :], start=True, stop=True)
                    act.copy(l4v[:, ci, :], lps[:, :])
                im0 = 32 * (qd // 3) + 4 * (qd % 3)
                sync.dma_start(
                    out=io["L4_d"][im0 : im0 + 4, 4 * chunk * CH : 4 * (chunk + 1) * CH],
                    in_=l4t[:],
                )

    qes.close()

    # ---------------- lerp corner logits + softmax + coef4 ----------------
    ces = contextlib.ExitStack()  # e_s lives: lerp .. coef4
    pe_pool = ces.enter_context(tc.tile_pool(name="pe_s", bufs=1))
    e_s = pe_pool.tile([64, NS], F32, name="e_s")
    with tc.tile_pool(name="lrp", bufs=1) as lrp:
        for qq in range(8):
            cs = slice(qq * QN, (qq + 1) * QN)
            l4 = lrp.tile([64, 4 * QN], F16, name="l4", tag="l4")
            act.memzero(l4[:])
            sync.dma_start(out=l4[0:12, :], in_=io["L4_d"][0:12, 4 * qq * QN : 4 * (qq + 1) * QN])
            sync.dma_start(out=l4[32:44, :], in_=io["L4_d"][32:44, 4 * qq * QN : 4 * (qq + 1) * QN])
            l4q = l4[:].rearrange("p (n four) -> p n four", four=4)
            ybl = lrp.tile([64, 2 * QN], F32, name="ybl", tag="ybl")
            tmp = lrp.tile([64, 2 * QN], F32, name="tmp", tag="tmp")
            dve.tensor_tensor(
                out=ybl[:].rearrange("p (n two) -> p n two", two=2),
                in0=l4q[:, :, 0:2],
                in1=btap(p_wy0[:, cs], QN, 2),
                op=AL.mult,
            )
            dve.tensor_tensor(
                out=tmp[:].rearrange("p (n two) -> p n two", two=2),
                in0=l4q[:, :, 2:4],
                in1=btap(p_wy1[:, cs], QN, 2),
                op=AL.mult,
            )
            dve.tensor_tensor(out=ybl[:], in0=ybl[:], in1=tmp[:], op=AL.add)
            dve.tensor_tensor(
                out=ybl[:],
                in0=ybl[:],
                in1=p_wxi[:, 2 * qq * QN : 2 * (qq + 1) * QN],
                op=AL.mult,
            )
            dve.tensor_reduce(
                out=e_s[:, cs],
                in_=ybl[:].rearrange("p (n two) -> p n two", two=2),
                axis=AX.X,
                op=AL.add,
            )
    with tc.tile_pool(name="smx", bufs=1) as smx:
        m9 = smx.tile([64, PX], F32, name="m9")
        dve.tensor_reduce(
            out=m9[:], in_=e_s[:].rearrange("p (n k) -> p n k", k=K), axis=AX.X, op=AL.max
        )
        msx = smx.tile([64, PX], F32, name="msx")
        act.memzero(msx[:])
        mt = smx.tile([12, PX], F32, name="mt")
        sync.dma_start(out=mt[:], in_=m9[32:44, :])
        dve.tensor_tensor(out=msx[0:12, :], in0=m9[0:12, :], in1=mt[:], op=AL.max)
        sync.dma_start(out=msx[32:44, :], in_=msx[0:12, :])
        dve.tensor_tensor(
            out=e_s[:].rearrange("p (n k) -> p n k", k=K),
            in0=e_s[:].rearrange("p (n k) -> p n k", k=K),
            in1=btap(msx[:], PX, K),
            op=AL.subtract,
        )
        act.activation(e_s[:], e_s[:], ACTF.Exp)
        s9 = smx.tile([64, PX], F32, name="s9")
        dve.tensor_reduce(
            out=s9[:], in_=e_s[:].rearrange("p (n k) -> p n k", k=K), axis=AX.X, op=AL.add
        )
        ssx = smx.tile([64, PX], F32, name="ssx")
        act.memzero(ssx[:])
        st = smx.tile([12, PX], F32, name="st")
        sync.dma_start(out=st[:], in_=s9[32:44, :])
        dve.tensor_tensor(out=ssx[0:12, :], in0=s9[0:12, :], in1=st[:], op=AL.add)
        dve.reciprocal(out=ssx[0:12, :], in_=ssx[0:12, :])
        sync.dma_start(out=ssx[32:44, :], in_=ssx[0:12, :])
        dve.tensor_tensor(
            out=e_s[:].rearrange("p (n k) -> p n k", k=K),
            in0=e_s[:].rearrange("p (n k) -> p n k", k=K),
            in1=btap(ssx[:], PX, K),
            op=AL.mult,
        )

    coef4_d = nc.dram_tensor("coef4_d", [64, 4 * NS], F16).ap()
    with tc.tile_pool(name="cfb", bufs=2) as cfb:
        for qq in range(8):
            cs = slice(qq * QN, (qq + 1) * QN)
            ca = cfb.tile([64, QN], F32, name="ca", tag="ca")
            cb = cfb.tile([64, QN], F32, name="cb", tag="cb")
            dve.tensor_tensor(out=ca[:], in0=e_s[:, cs], in1=p_wy0[:, cs], op=AL.mult)
            dve.tensor_tensor(out=cb[:], in0=e_s[:, cs], in1=p_wy1[:, cs], op=AL.mult)
            c4t = cfb.tile([64, 4 * QN], F16, name="c4t", tag="c4t")
            c4 = c4t[:].rearrange("p (n four) -> p n four", four=4)
            wxi = p_wxi[:, 2 * qq * QN : 2 * (qq + 1) * QN].rearrange(
                "p (n two) -> p n two", two=2
            )
            dve.tensor_tensor(out=c4[:, :, 0:2], in0=btap(ca[:], QN, 2), in1=wxi, op=AL.mult)
            dve.tensor_tensor(out=c4[:, :, 2:4], in0=btap(cb[:], QN, 2), in1=wxi, op=AL.mult)
            sync.dma_start(out=coef4_d[:, 4 * qq * QN : 4 * (qq + 1) * QN], in_=c4t[:])
    ces.close()
    wes.close()

    # ---------------- V phase ----------------
    with (
        tc.tile_pool(name="vcv", bufs=1) as vcv,
        tc.tile_pool(name="vsc", bufs=2) as vsc,
        tc.tile_pool(name="vpp", bufs=1, space="PSUM") as vpp,
        tc.tile_pool(name="vpp2", bufs=2, space="PSUM") as vpp2,
    ):
        for qd3 in range(3):
            otmp = [
                vsc.tile([24, PX], F32, name=f"otmp{j}", tag=f"otmp{j}") for j in range(4)
            ]
            for ti in range(T):
                qd = 3 * ti + qd3
                canq = make_canvas(vcv, vsc, vpp2, 1, qd)
                red = vsc.tile([128, PX], F32, name="red", tag="red")
                for chunk in range(NCH):
                    wsl = wrp[
                        :, qd * (NS // 16) + chunk * NW : qd * (NS // 16) + (chunk + 1) * NW
                    ]
                    mall = vsc.tile([128, 4 * CH], F32, name="mall", tag="mall")
                    mallv = mall[:].rearrange("p (n four k) -> p n four k", four=4, k=K)
                    cft = vsc.tile([64, 4 * CH], F16, name="cft", tag="cft")
                    sync.dma_start(
                        out=cft[:], in_=coef4_d[:, 4 * chunk * CH : 4 * (chunk + 1) * CH]
                    )
                    cfv = cft[:].rearrange("p (n four) -> p four n", four=4)
                    for ci, dlt in enumerate((0, 1, CC, CC + 1)):
                        it = vsc.tile([128, NW], I16, name="vit", tag="vit")
                        dve.tensor_scalar(
                            out=it[:], in0=wsl, scalar1=dlt, scalar2=None, op0=AL.add
                        )
                        gt = vsc.tile([128, CH], F32, name="vgt", tag="vgt")
                        gps.ap_gather(
                            gt[:], canq[:].unsqueeze(-1), it[:], 128, CN, 1, CH
                        )
                        crp = vpp.tile([128, CH], F32, name="crp", tag="crp")
                        mm(
                            crp[:, :],
                            selrep_s[32 * ti : 32 * ti + 12, qd3 * 128 : qd3 * 128 + 128],
                            cfv[32 * ti : 32 * ti + 12, ci, :],
                            start=True,
                            stop=True,
                        )
                        dve.tensor_tensor(
                            out=mallv[:, :, ci, :],
                            in0=gt[:].rearrange("p (n k) -> p n k", k=K),
                            in1=crp[:, :].rearrange("p (n k) -> p n k", k=K),
                            op=AL.mult,
                        )
                    dve.tensor_reduce(
                        out=red[:, chunk * CHPX : (chunk + 1) * CHPX],
                        in_=mall[:].rearrange("p (n fk) -> p n fk", fk=4 * K),
                        axis=AX.X,
                        op=AL.add,
                    )
                redb = vsc.tile([128, PX], BF, name="redb", tag="redb")
                dve.tensor_copy(out=redb[:], in_=red[:])
                for j in range(4):
                    vt = vpp2.tile([24, PX], F32, name="vt", tag="vt")
                    mm(
                        vt[:, :],
                        selv4_s[:, HD * j : HD * j + HD],
                        redb[:, :],
                        start=True,
                        stop=True,
                    )
                    if ti == 0:
                        dve.tensor_copy(out=otmp[j][:], in_=vt[:, :])
                    else:
                        dve.tensor_tensor(out=otmp[j][:], in0=otmp[j][:], in1=vt[:, :], op=AL.add)
            for j in range(4):
                g = 4 * qd3 + j
                _dma_to_chrows(sync, oatt, PX, otmp[j][:], 24 * g)

    sync.dma_start(out=io["oatt_d"][:], in_=oatt[:])

    # ---------------- MLP (exact gelu) + residual ----------------
    with (
        tc.tile_pool(name="mlp", bufs=2) as mp,
        tc.tile_pool(name="mlps", bufs=1) as mps,
        tc.tile_pool(name="mpp", bufs=2, space="PSUM") as mpp,
    ):
        oattb = mps.tile([128, 3 * PX], BF, name="oattb")
        dve.tensor_copy(out=oattb[:], in_=oatt[:])
        w1_s = mps.tile([128, 3 * 2 * C], BF, name="w1_s")
        w2_s = mps.tile([128, 5 * C], BF, name="w2_s")
        b1_s = mps.tile([128, 5], F32, name="b1_s")
        b2_s = mps.tile([128, 3], F32, name="b2_s")
        h_s = mps.tile([128, 5 * PX], BF, name="h_s")
        for i in range(3):
            n = min(128, C - 128 * i)
            sync.dma_start(
                out=w1_s[:n, i * 2 * C : (i + 1) * 2 * C],
                in_=io["w1t"][128 * i : 128 * i + n, :],
            )
            sync.dma_start(out=b2_s[:n, i : i + 1], in_=io["b2"][128 * i : 128 * i + n, :])
        for i in range(5):
            n = min(128, 2 * C - 128 * i)
            sync.dma_start(out=w2_s[:n, i * C : (i + 1) * C], in_=io["w2t"][128 * i : 128 * i + n, :])
            sync.dma_start(out=b1_s[:n, i : i + 1], in_=io["b1"][128 * i : 128 * i + n, :])
        for m in range(5):
            mn = min(128, 2 * C - 128 * m)
            for nch in range(PX // 512):
                ps = mpp.tile([128, 512], F32, name="m1ps", tag="m1ps")
                for kk in range(3):
                    kn = min(128, C - 128 * kk)
                    mm(
                        ps[:mn, :],
                        w1_s[:kn, kk * 2 * C + 128 * m : kk * 2 * C + 128 * m + mn],
                        oattb[:kn, kk * PX + nch * 512 : kk * PX + nch * 512 + 512],
                        start=(kk == 0),
                        stop=(kk == 2),
                    )
                xg = mp.tile([128, 512], F32, name="xg", tag="xg")
                dve.tensor_tensor(
                    out=xg[:mn, :],
                    in0=ps[:mn, :],
                    in1=b1_s[:mn, m : m + 1].to_broadcast([mn, 512]),
                    op=AL.add,
                )
                er = mp.tile([128, 512], F32, name="er", tag="er")
                act.activation(
                    er[:mn, :], xg[:mn, :], ACTF.Erf, bias=0.0, scale=0.7071067811865476
                )
                dve.tensor_scalar(
                    out=er[:mn, :], in0=er[:mn, :], scalar1=1.0, scalar2=0.5, op0=AL.add, op1=AL.mult
                )
                dve.tensor_tensor(
                    out=h_s[:mn, m * PX + nch * 512 : m * PX + nch * 512 + 512],
                    in0=xg[:mn, :],
                    in1=er[:mn, :],
                    op=AL.mult,
                )
        for m in range(3):
            mn = min(128, C - 128 * m)
            for nch in range(PX // 512):
                ps = mpp.tile([128, 512], F32, name="m2ps", tag="m2ps")
                for kk in range(5):
                    kn = min(128, 2 * C - 128 * kk)
                    mm(
                        ps[:mn, :],
                        w2_s[:kn, kk * C + 128 * m : kk * C + 128 * m + mn],
                        h_s[:kn, kk * PX + nch * 512 : kk * PX + nch * 512 + 512],
                        start=(kk == 0),
                        stop=(kk == 4),
                    )
                og = mp.tile([128, 512], F32, name="og", tag="og")
                dve.tensor_tensor(
                    out=og[:mn, :],
                    in0=ps[:mn, :],
                    in1=b2_s[:mn, m : m + 1].to_broadcast([mn, 512]),
                    op=AL.add,
                )
                dve.tensor_tensor(
                    out=og[:mn, :],
                    in0=og[:mn, :],
                    in1=oatt[:mn, m * PX + nch * 512 : m * PX + nch * 512 + 512],
                    op=AL.add,
                )
                sync.dma_start(
                    out=io["out_d"][128 * m : 128 * m + mn, nch * 512 : nch * 512 + 512],
                    in_=og[:mn, :],
                )
    es.close()


# ============================ host side ============================


def _host_inputs(q, k, v, offset, Wq, bq, Wk, bk, Wv, bv, W1, b1, W2, b2):
    cores = []
    shared = {}
    BF_np = ml_dtypes.bfloat16
    shared["wqt"] = np.ascontiguousarray(np.asarray(Wq).T).astype(BF_np)
    shared["wkt"] = np.ascontiguousarray(np.asarray(Wk).T).astype(BF_np)
    shared["wvt"] = np.ascontiguousarray(np.asarray(Wv).T).astype(BF_np)
    shared["w1t"] = np.ascontiguousarray(np.asarray(W1).T).astype(BF_np)
    shared["w2t"] = np.ascontiguousarray(np.asarray(W2).T).astype(BF_np)
    shared["bqs"] = (np.asarray(bq) * SCALE).reshape(C, 1).astype(np.float32)
    bkvq = np.zeros((96, 6), np.float32)
    for qd3 in range(3):
        bkvq[:, 0 * 3 + qd3] = np.asarray(bk)[96 * qd3 : 96 * qd3 + 96]
        bkvq[:, 1 * 3 + qd3] = np.asarray(bv)[96 * qd3 : 96 * qd3 + 96]
    shared["bkvq"] = bkvq
    shared["b1"] = np.asarray(b1).reshape(2 * C, 1).astype(np.float32)
    shared["b2"] = np.asarray(b2).reshape(C, 1).astype(np.float32)
    sel4 = np.zeros((128, 4), ml_dtypes.bfloat16)
    for j in range(4):
        sel4[32 * j : 32 * j + 24, j] = 1.0
    shared["sel4"] = sel4
    selrep = np.zeros((64, 3 * 128), np.float16)
    for ti in range(2):
        for qd3 in range(3):
            for p in range(128):
                selrep[32 * ti + 4 * qd3 + p // 32, qd3 * 128 + p] = 1.0
    shared["selrep"] = selrep
    cores = []
    KH = KW = 3
    offr = np.asarray(offset).reshape(B, T, G, KH * KW, 2, H, W)
    for core in range(8):
        b, R0 = core // 4, 16 * (core % 4)
        d = dict(shared)
        d["q_in"] = np.ascontiguousarray(
            np.asarray(q)[b, 0, :, R0 : R0 + RB, :].reshape(C, PX)
        ).astype(ml_dtypes.bfloat16)
        for name, src in (("k_in", k), ("v_in", v)):
            halo = np.zeros((T, C, HALO, W), np.float32)
            lo, hi = R0 - 16, R0 + 32
            slo, shi = max(lo, 0), min(hi, H)
            halo[:, :, slo - lo : shi - lo, :] = np.asarray(src)[b, :, :, slo:shi, :]
            d[name] = np.ascontiguousarray(halo.reshape(T, C, HALO * W)).astype(ml_dtypes.bfloat16)
        off = offr[b, :, :, :, :, R0 : R0 + RB, :]  # (T,G,K,2,RB,W)
        offp = np.zeros((128, NS), np.float32)
        basep = np.zeros((128, NS), np.float32)
        ky = np.repeat(np.arange(KH), KW).astype(np.float32)
        kx = np.tile(np.arange(KW), KH).astype(np.float32)
        py = (R0 + np.arange(RB, dtype=np.float32))[:, None, None]
        pxc = np.arange(W, dtype=np.float32)[None, :, None]
        base_y = np.broadcast_to(py + (ky[None, None, :] - 1.0) + 64.0, (RB, W, K))
        base_x = np.broadcast_to(pxc + (kx[None, None, :] - 1.0) + 64.0, (RB, W, K))
        for t in range(T):
            for g in range(G):
                r = t * 32 + g
                offp[r] = off[t, g, :, 0].transpose(1, 2, 0).reshape(NS)
                offp[64 + r] = off[t, g, :, 1].transpose(1, 2, 0).reshape(NS)
                basep[r] = base_y.reshape(NS)
                basep[64 + r] = base_x.reshape(NS)
        d["off_in"] = offp
        d["base_in"] = basep
        csub = np.zeros((128, 1), np.float32)
        csub[:64] = 64.0 + R0 - 16.0
        csub[64:] = 64.0 - 16.0
        d["csub_in"] = csub
        chi = np.zeros((128, 1), np.float32)
        chi[:64] = 47.0
        chi[64:] = 95.0
        d["chi_in"] = chi
        selv4 = np.zeros((128, 4 * HD), ml_dtypes.bfloat16)
        for j in range(4):
            for dd in range(HD):
                selv4[32 * j + dd, HD * j + dd] = 1.0
        d["selv4"] = selv4
        cores.append(d)
    return cores


def kernel(q, k, v, offset, Wq, bq, Wk, bk, Wv, bv, W1, b1, W2, b2):
    if "nc" not in _CACHE:
        _CACHE["nc"] = build_program()
    nc = _CACHE["nc"]
    ins = _host_inputs(q, k, v, offset, Wq, bq, Wk, bk, Wv, bv, W1, b1, W2, b2)
    res = run_bass_kernel_spmd(nc, ins, list(range(8))).results
    out = np.zeros((B, 1, C, H, W), np.float32)
    for core in range(8):
        b, R0 = core // 4, 16 * (core % 4)
        out[b, 0, :, R0 : R0 + RB, :] = res[core]["out"].reshape(C, RB, W)
    return out

